# revision 46
# baseline (speedup 1.0000x reference)
"""Trainium2 Bass kernel for nn_LungCancerGRU (GRU H=64, T=15, B=262144 -> logits [B,2]).

Data parallel over 8 NeuronCores (batch sharded, 32768 rows/core).

Per-core layout: gate units on SBUF partitions, batch on the free dimension.
Batch runs in pair-tiles of 1024 rows = two groups (A, B) of N=512; group A
occupies partitions 0..63, group B 64..127 of every [128, 512] tile.

IL=4 pair-chains run interleaved to hide the recurrence dependency cycle
(sigmoid -> m1 -> accumulate -> tanh -> h').  Each chain needs only 2 PSUM
banks: the r and z preacts share one bank sequentially (z is issued after
the r sigmoid frees it), and the n-gate bank is reused within the step (the
x_n matmul's start=True reset happens after the scalar_tensor_tensor has
consumed hgn from the same bank).  4 chains x 2 banks = the full 8 banks.

All matmuls are bf16 (fp32 moving data costs 4 cycles/column; strided bf16
costs 2).  x is cast to bf16 once in a wide layout with a transposing DVE
view and round-tripped through scratch DRAM laid out t-major, so per-pair
loads are 2 contiguous descriptors and per-step rhs slices are contiguous.

The x lhsT blocks are K=128 zero-padded (rows 0/1 = per-group w_in masks,
row 2 = gate bias against a ones row in the xt tile, rows 3..127 = 0): low-K
matmuls starve the PE activity monitor and throttle the PE clock to half
rate; zero-padding keeps the array duty high and the clock at full speed.

Per chain and timestep t:
  p_r  = x2_r @ xt + BD(W_hr^T) @ h;  r = sigmoid(p_r)     [bank A]
  p_n  = BD(W_hn^T) @ h                                    [bank B]
  p_z  = x2_z @ xt + BD(W_hz^T) @ h;  z = sigmoid(p_z)     [bank A again]
  m1   = (p_n + b_hh_n) * r          DVE scalar_tensor_tensor
  p_n  = x2_n @ xt (start=True reset) + I128 @ m1          [bank B again]
  n    = tanh(p_n)
  w = 1-z; zh = z*h; nw = n*w; h' = nw + zh                DVE bf16

FC head: logitsT [2, 512] per group via PE (stationary W_fc^T slice), bias
added in the PSUM->SBUF tensor_scalar copy, staged per-superblock and DMA'd
to a transposed [2, BC] bf16 DRAM output; the host transposes back.
"""

import sys

import numpy as np

sys.path.insert(0, "/opt/trn_rl_repo")

B, T, H = 262144, 15, 64
NCORES = 8
BC = B // NCORES          # 32768 rows per core
N = 512                   # batch columns per group
PAIR = 2 * N              # 1024 rows per pair-tile
NPAIR = BC // PAIR        # 32 pair-tiles per core
IL = 4                    # pair-tiles processed in lockstep
XW = T * N                # xt tile free width (7680)
SB = IL * PAIR            # rows per superblock (2048)

_cache = {}


def _build():
    from contextlib import ExitStack

    import concourse.bacc as bacc
    import concourse.mybir as mybir
    from concourse.tile import TileContext

    f32 = mybir.dt.float32
    bf16 = mybir.dt.bfloat16
    Act = mybir.ActivationFunctionType
    Alu = mybir.AluOpType

    nc = bacc.Bacc(None)

    x_in = nc.dram_tensor("x", [BC, T], f32, kind="ExternalInput")
    out_d = nc.dram_tensor("out", [2, BC], bf16, kind="ExternalOutput")
    cbf_in = nc.dram_tensor("cbf", [128, 1024], bf16, kind="ExternalInput")
    ones_in = nc.dram_tensor("ones", [1, XW], bf16, kind="ExternalInput")
    cf_in = nc.dram_tensor("cf", [128, 8], f32, kind="ExternalInput")
    # scratch x, bf16, t-major per 512-row group-block: xs[r, t*512+n] = x[512r+n, t]
    xs_d = nc.dram_tensor("xs", [BC // N, XW], bf16, kind="Internal")

    with TileContext(nc) as tc, ExitStack() as es:
        # ---- constants ----
        cpool = es.enter_context(tc.tile_pool(name="const", bufs=1))
        cbf = cpool.tile([128, 1024], bf16)
        nc.sync.dma_start(cbf[:], cbf_in[:])
        cf = cpool.tile([128, 8], f32)
        nc.sync.dma_start(cf[:], cf_in[:])

        bd_g = [cbf[:, 128 * g:128 * (g + 1)] for g in range(3)]
        i128 = cbf[:, 384:512]
        wfc = cbf[:, 512:514]
        # full-height x lhsT: row0/1 = per-group w_in masks, row2 = bias,
        # rows 3..127 zero.  K=128 keeps the PE array duty high (HAM stays
        # at full clock; low-K matmuls make it throttle).
        x2_g = [cbf[:, 514 + 128 * g:514 + 128 * (g + 1)] for g in range(3)]
        b2 = cf[:, 3:4]
        bfc = cf[0:2, 4:5]

        # ---- x pre-pass: f32 wide load -> transposing bf16 cast -> scratch ----
        xpre = es.enter_context(tc.tile_pool(name="xpre", bufs=1))
        xw = xpre.tile([64, XW], f32)
        nc.sync.dma_start(xw[:], x_in[:].rearrange("(p n) t -> p (n t)", p=64))
        xbw = xpre.tile([64, XW], bf16)
        nc.vector.tensor_copy(xbw[:].rearrange("p (t n) -> p t n", n=N),
                              xw[:].rearrange("p (n t) -> p t n", t=T))
        nc.sync.dma_start(xs_d[:], xbw[:])

        # ---- pools ----
        xt_pool = es.enter_context(tc.tile_pool(name="xt", bufs=5))
        # prime the xt buffers once: rows 2..127 never rewritten in-loop
        # (DMA fills rows 0..1 only); row 2 = ones carries the bias rows.
        for i in range(5):
            xtp = xt_pool.tile([128, XW], bf16, tag="xt", name=f"xtprime{i}")
            nc.gpsimd.memset(xtp[:], 0.0)
            nc.sync.dma_start(xtp[2:3, :], ones_in[:])
        hp = es.enter_context(tc.tile_pool(name="h", bufs=IL + 2))
        rzp = es.enter_context(tc.tile_pool(name="rz", bufs=IL + 2))
        zp = es.enter_context(tc.tile_pool(name="z", bufs=IL + 2))
        m1p = es.enter_context(tc.tile_pool(name="m1", bufs=IL + 2))
        np_ = es.enter_context(tc.tile_pool(name="nt", bufs=IL + 2))
        wp = es.enter_context(tc.tile_pool(name="w", bufs=IL + 2))
        zhp = es.enter_context(tc.tile_pool(name="zh", bufs=IL + 2))
        nwp = es.enter_context(tc.tile_pool(name="nw", bufs=IL + 2))
        stp = es.enter_context(tc.tile_pool(name="stage", bufs=2))
        prz = es.enter_context(tc.tile_pool(name="prz", bufs=4, space="PSUM"))
        pn = es.enter_context(tc.tile_pool(name="pn", bufs=4, space="PSUM"))
        plog = pn  # FC logits rotate through the pn slots (shared tag)

        def mm(out, lhsT, rhs, start, stop):
            nc.tensor.matmul(out, lhsT, rhs, start=start, stop=stop,
                             skip_group_check=True)

        # ---- engine warm-ups: fold const-DMA sems into each engine's clock
        pwarm = plog.tile([2, 2], f32, tag="pn")
        mm(pwarm[:], cbf[0:2, 0:2], cbf[0:2, 0:2], True, True)
        wt = cpool.tile([2, 8], f32)
        nc.vector.tensor_copy(wt[0:1, 0:1], cf[0:1, 0:1])
        nc.vector.tensor_copy(wt[0:1, 1:2], cbf[0:1, 0:1])
        nc.scalar.copy(wt[0:1, 2:3], cf[0:1, 0:1])
        nc.scalar.copy(wt[0:1, 3:4], cbf[0:1, 0:1])

        def stage_r(pr, t):
            """r gate: x matmul (start) + recurrent matmul (stop), sigmoid.
            One 1-bank psum tile, freed at the sigmoid -> z reuses it."""
            xcols = pr["xtv"][:, t, :]
            pr["xc"] = xcols
            p_r = prz.tile([128, N], f32, tag="prz")
            h = pr["h"]
            mm(p_r[:], x2_g[0], xcols, True, h is None)
            if h is not None:
                mm(p_r[:], bd_g[0], h[:], False, True)
            r_t = rzp.tile([128, N], bf16, tag="rz")
            nc.scalar.activation(r_t[:], p_r[:], Act.Sigmoid)
            pr["r_t"] = r_t

        def stage_n1(pr, t):
            """hgn = W_hn h into the n-gate psum bank (reused below)."""
            h = pr["h"]
            p_n = pn.tile([128, N], f32, tag="pn")
            if h is not None:
                mm(p_n[:], bd_g[2], h[:], True, True)
            pr["p_n"] = p_n

        def stage_z(pr, t):
            """z gate in the bank stage_r freed."""
            p_z = prz.tile([128, N], f32, tag="prz")
            h = pr["h"]
            mm(p_z[:], x2_g[1], pr["xc"], True, h is None)
            if h is not None:
                mm(p_z[:], bd_g[1], h[:], False, True)
            z_t = zp.tile([128, N], bf16, tag="z")
            nc.scalar.activation(z_t[:], p_z[:], Act.Sigmoid)
            pr["z_t"] = z_t

        def stage_m(pr, t):
            """m1 = (hgn + b_hh_n)*r; x_n start=True resets the same bank;
            identity matmul accumulates m1; tanh."""
            p_n, r_t = pr["p_n"], pr["r_t"]
            m1 = m1p.tile([128, N], bf16, tag="m1")
            if pr["h"] is not None:
                nc.vector.scalar_tensor_tensor(m1[:], p_n[:], b2, r_t[:],
                                               Alu.add, Alu.mult)
            else:
                nc.vector.tensor_scalar(m1[:], r_t[:], b2, None, Alu.mult)
            mm(p_n[:], x2_g[2], pr["xc"], True, False)
            mm(p_n[:], i128, m1[:], False, True)
            n_t = np_.tile([128, N], bf16, tag="nt")
            nc.scalar.activation(n_t[:], p_n[:], Act.Tanh)
            pr["n_t"] = n_t

        def stage_c(pr, t):
            """h' = n + z*(h - n)  (3 DVE ops; t=0: h' = n*(1-z))."""
            n_t, z_t, h = pr["n_t"], pr["z_t"], pr["h"]
            h_new = hp.tile([128, N], bf16, tag="h")
            if h is not None:
                d = zhp.tile([128, N], bf16, tag="zh")
                nc.vector.tensor_tensor(d[:], h[:], n_t[:], Alu.subtract)
                e = nwp.tile([128, N], bf16, tag="nw")
                nc.vector.tensor_tensor(e[:], z_t[:], d[:], Alu.mult)
                nc.vector.tensor_tensor(h_new[:], n_t[:], e[:], Alu.add)
            else:
                w = wp.tile([128, N], bf16, tag="w")
                nc.vector.tensor_scalar(w[:], z_t[:], -1.0, 1.0,
                                        Alu.mult, Alu.add)
                nc.vector.tensor_tensor(h_new[:], n_t[:], w[:], Alu.mult)
            pr["h"] = h_new

        def fc_out(pr, st, j):
            h = pr["h"]
            for g in range(2):
                p_l = plog.tile([2, N], f32, tag="pn")
                mm(p_l[:], wfc[64 * g:64 * (g + 1), :], h[64 * g:64 * (g + 1), :],
                   True, True)
                stg = st[0:2, j * PAIR + g * N:j * PAIR + (g + 1) * N]
                nc.vector.tensor_scalar(stg, p_l[:], bfc, None, Alu.add)

        for blk in range(NPAIR // IL):
            sbbase = blk * SB
            pairs = []
            st = stp.tile([2, SB], bf16, tag="st")
            for j in range(IL):
                pidx = blk * IL + j
                base = sbbase + j * PAIR
                xt = xt_pool.tile([128, XW], bf16, tag="xt")
                # flat contiguous DMA: 2 descriptors of 15KB
                nc.sync.dma_start(xt[0:2, :], xs_d[2 * pidx:2 * pidx + 2, :])
                pairs.append({"xtv": xt[:].rearrange("g (t n) -> g t n", n=N),
                              "base": base, "h": None})
            for t in range(T):
                for pr in pairs:
                    stage_r(pr, t)
                for pr in pairs:
                    stage_n1(pr, t)
                for pr in pairs:
                    stage_z(pr, t)
                for pr in pairs:
                    stage_m(pr, t)
                for pr in pairs:
                    stage_c(pr, t)
            for j, pr in enumerate(pairs):
                fc_out(pr, st, j)
            nc.sync.dma_start(out_d[0:2, sbbase:sbbase + SB], st[0:2, :])

    nc.compile()
    return nc


def _host_constants(W_ih, W_hh, b_ih, b_hh, W_fc, b_fc):
    import ml_dtypes

    f32 = np.float32
    cbf = np.zeros((128, 1024), f32)
    cf = np.zeros((128, 8), f32)
    w_in = W_ih[:, 0].astype(f32)
    bias_g = [
        b_ih[0:64] + b_hh[0:64],          # r
        b_ih[64:128] + b_hh[64:128],      # z
        b_ih[128:192],                    # n (b_hh_n applied inside r* via b2)
    ]
    for g in range(3):
        W = W_hh[64 * g:64 * (g + 1)].astype(f32)          # [64, 64]
        cbf[0:64, 128 * g:128 * g + 64] = W.T
        cbf[64:128, 128 * g + 64:128 * g + 128] = W.T
        wg = w_in[64 * g:64 * (g + 1)]
        cbf[0, 514 + 128 * g:514 + 128 * g + 64] = wg
        cbf[1, 514 + 128 * g + 64:514 + 128 * g + 128] = wg
        cbf[2, 514 + 128 * g:514 + 128 * (g + 1)] = np.concatenate([bias_g[g]] * 2)
    cbf[:, 384:512] = np.eye(128, dtype=f32)
    cbf[0:64, 512:514] = W_fc.T
    cbf[64:128, 512:514] = W_fc.T
    cf[:, 3] = np.concatenate([b_hh[128:192]] * 2)
    cf[0:2, 4] = b_fc
    return {"cbf": cbf.astype(ml_dtypes.bfloat16), "cf": cf}


def kernel(x, W_ih, W_hh, b_ih, b_hh, W_fc, b_fc, _trace=False, _trace_kwargs=None):
    from concourse.bass_utils import run_bass_kernel_spmd

    if "nc" not in _cache:
        _cache["nc"] = _build()
    nc = _cache["nc"]

    consts = _host_constants(W_ih, W_hh, b_ih, b_hh, W_fc, b_fc)
    import ml_dtypes
    consts["ones"] = np.ones((1, XW), dtype=ml_dtypes.bfloat16)
    x = np.ascontiguousarray(np.asarray(x, np.float32))
    in_maps = []
    for c in range(NCORES):
        m = {"x": x[c * BC:(c + 1) * BC]}
        m.update(consts)
        in_maps.append(m)
    kw = {}
    if _trace:
        kw["trace"] = True
        if _trace_kwargs:
            kw.update(_trace_kwargs)
    res = run_bass_kernel_spmd(nc, in_maps, list(range(NCORES)), **kw)
    out = np.concatenate(
        [np.asarray(res.results[c]["out"]).astype(np.float32).T
         for c in range(NCORES)], axis=0)
    if _trace:
        return out, res
    return out


if __name__ == "__main__":
    rng = np.random.default_rng(0)
    s = 1.0 / np.sqrt(H)
    inputs = {
        "x": rng.standard_normal((B, T), dtype=np.float32),
        "W_ih": rng.uniform(-s, s, (3 * H, 1)).astype(np.float32),
        "W_hh": rng.uniform(-s, s, (3 * H, H)).astype(np.float32),
        "b_ih": rng.uniform(-s, s, (3 * H,)).astype(np.float32),
        "b_hh": rng.uniform(-s, s, (3 * H,)).astype(np.float32),
        "W_fc": rng.uniform(-s, s, (2, H)).astype(np.float32),
        "b_fc": rng.uniform(-s, s, (2,)).astype(np.float32),
    }
    out = kernel(**inputs)
    print(out.shape, out.dtype, out[:4])


# revision 47
# speedup vs baseline: 1.0521x; 1.0521x over previous
"""Trainium2 Bass kernel for nn_LungCancerGRU (GRU H=64, T=15, B=262144 -> logits [B,2]).

Data parallel over 8 NeuronCores (batch sharded, 32768 rows/core).

Per-core layout: gate units on SBUF partitions, batch on the free dimension.
Batch runs in pair-tiles of 1024 rows = two groups (A, B) of N=512; group A
occupies partitions 0..63, group B 64..127 of every [128, 512] tile.  Two
pair-tiles (IL=2) run in lockstep to hide the recurrence critical path.

All matmuls are bf16 (moving operand dtype determines PE rate; fp32 moving
data costs 4 cycles/column).  x is cast to bf16 once in a wide 128-partition
layout and round-tripped through scratch DRAM so the per-pair transposed
loads are 2 contiguous descriptors instead of 1024 60-byte ones.

Per timestep t (per pair-tile):
  p_rz[:, :512] = BD(W_hr^T) @ h + x2_r @ x_t       (r preact, K=128 + K=2)
  p_rz[:, 512:] = BD(W_hz^T) @ h + x2_z @ x_t       (z preact)
  p_hgn         = BD(W_hn^T) @ h                    (h-part of n gate)
  p_n           = x2_n @ x_t                        (x-part of n gate)
  r   = sigmoid(p_rz[:, :512] + bias_r)             ACT, per-partition bias
  z   = sigmoid(p_rz[:, 512:] + bias_z)
  m1  = (p_hgn + b_hh_n) * r                        DVE scalar_tensor_tensor
  p_n += I128 @ m1                                  identity-matmul accumulate
  n   = tanh(p_n + b_ih_n)                          ACT
  w = 1-z; zh = z*h; nw = n*w; h' = nw + zh         DVE bf16

FC head: logitsT [2, 512] per group via PE (stationary W_fc^T slice), bias
added in the PSUM->SBUF tensor_scalar copy, staged per-superblock and DMA'd
to a transposed [2, BC] bf16 DRAM output; the host transposes back.
"""

import sys

import numpy as np

sys.path.insert(0, "/opt/trn_rl_repo")

B, T, H = 262144, 15, 64
NCORES = 8
BC = B // NCORES          # 32768 rows per core
N = 512                   # batch columns per group
PAIR = 2 * N              # 1024 rows per pair-tile
NPAIR = BC // PAIR        # 32 pair-tiles per core
IL = 4                    # pair-tiles processed in lockstep
XW = T * N                # xt tile free width (7680)
SB = IL * PAIR            # rows per superblock (2048)

_cache = {}


def _build():
    from contextlib import ExitStack

    import concourse.bacc as bacc
    import concourse.mybir as mybir
    from concourse.tile import TileContext

    f32 = mybir.dt.float32
    bf16 = mybir.dt.bfloat16
    Act = mybir.ActivationFunctionType
    Alu = mybir.AluOpType

    nc = bacc.Bacc(None)

    x_in = nc.dram_tensor("x", [BC, T], f32, kind="ExternalInput")
    out_d = nc.dram_tensor("out", [2, BC], bf16, kind="ExternalOutput")
    cbf_in = nc.dram_tensor("cbf", [128, 1024], bf16, kind="ExternalInput")
    ones_in = nc.dram_tensor("ones", [1, XW], bf16, kind="ExternalInput")
    cf_in = nc.dram_tensor("cf", [128, 8], f32, kind="ExternalInput")
    # scratch x, bf16, t-major per 512-row group-block: xs[r, t*512+n] = x[512r+n, t]
    xs_d = nc.dram_tensor("xs", [BC // N, XW], bf16, kind="Internal")

    with TileContext(nc) as tc, ExitStack() as es:
        # ---- constants ----
        cpool = es.enter_context(tc.tile_pool(name="const", bufs=1))
        cbf = cpool.tile([128, 1024], bf16)
        nc.sync.dma_start(cbf[:], cbf_in[:])
        cf = cpool.tile([128, 8], f32)
        nc.sync.dma_start(cf[:], cf_in[:])

        bd_g = [cbf[:, 128 * g:128 * (g + 1)] for g in range(3)]
        i128 = cbf[:, 384:512]
        wfc = cbf[:, 512:514]
        # full-height x lhsT: row0/1 = per-group w_in masks, row2 = bias,
        # rows 3..127 zero.  K=128 keeps the PE array duty high (HAM stays
        # at full clock; low-K matmuls make it throttle).
        x2_g = [cbf[:, 514 + 128 * g:514 + 128 * (g + 1)] for g in range(3)]
        b2 = cf[:, 3:4]
        bfc = cf[0:2, 4:5]

        # ---- x pre-pass: f32 wide load -> transposing bf16 cast -> scratch ----
        xpre = es.enter_context(tc.tile_pool(name="xpre", bufs=1))
        xw = xpre.tile([64, XW], f32)
        nc.sync.dma_start(xw[:], x_in[:].rearrange("(p n) t -> p (n t)", p=64))
        xbw = xpre.tile([64, XW], bf16)
        nc.vector.tensor_copy(xbw[:].rearrange("p (t n) -> p t n", n=N),
                              xw[:].rearrange("p (n t) -> p t n", t=T))
        nc.sync.dma_start(xs_d[:], xbw[:])

        # ---- pools ----
        xt_pool = es.enter_context(tc.tile_pool(name="xt", bufs=5))
        # prime the xt buffers once: rows 2..127 never rewritten in-loop
        # (DMA fills rows 0..1 only); row 2 = ones carries the bias rows.
        for i in range(5):
            xtp = xt_pool.tile([128, XW], bf16, tag="xt", name=f"xtprime{i}")
            nc.gpsimd.memset(xtp[:], 0.0)
            nc.sync.dma_start(xtp[2:3, :], ones_in[:])
        hp = es.enter_context(tc.tile_pool(name="h", bufs=IL + 4))
        rzp = es.enter_context(tc.tile_pool(name="rz", bufs=IL + 4))
        zp = es.enter_context(tc.tile_pool(name="z", bufs=IL + 4))
        m1p = es.enter_context(tc.tile_pool(name="m1", bufs=IL + 4))
        np_ = es.enter_context(tc.tile_pool(name="nt", bufs=IL + 4))
        wp = es.enter_context(tc.tile_pool(name="w", bufs=IL + 2))
        zhp = es.enter_context(tc.tile_pool(name="zh", bufs=IL + 2))
        nwp = es.enter_context(tc.tile_pool(name="nw", bufs=IL + 2))
        stp = es.enter_context(tc.tile_pool(name="stage", bufs=2))
        prz = es.enter_context(tc.tile_pool(name="prz", bufs=4, space="PSUM"))
        pn = es.enter_context(tc.tile_pool(name="pn", bufs=4, space="PSUM"))
        plog = pn  # FC logits rotate through the pn slots (shared tag)

        def mm(out, lhsT, rhs, start, stop):
            nc.tensor.matmul(out, lhsT, rhs, start=start, stop=stop,
                             skip_group_check=True)

        # ---- engine warm-ups: fold const-DMA sems into each engine's clock
        pwarm = plog.tile([2, 2], f32, tag="pn")
        mm(pwarm[:], cbf[0:2, 0:2], cbf[0:2, 0:2], True, True)
        wt = cpool.tile([2, 8], f32)
        nc.vector.tensor_copy(wt[0:1, 0:1], cf[0:1, 0:1])
        nc.vector.tensor_copy(wt[0:1, 1:2], cbf[0:1, 0:1])
        nc.scalar.copy(wt[0:1, 2:3], cf[0:1, 0:1])
        nc.scalar.copy(wt[0:1, 3:4], cbf[0:1, 0:1])

        def stage_r(pr, t):
            """r gate: x matmul (start) + recurrent matmul (stop), sigmoid.
            One 1-bank psum tile, freed at the sigmoid -> z reuses it."""
            xcols = pr["xtv"][:, t, :]
            pr["xc"] = xcols
            p_r = prz.tile([128, N], f32, tag="prz")
            h = pr["h"]
            mm(p_r[:], x2_g[0], xcols, True, h is None)
            if h is not None:
                mm(p_r[:], bd_g[0], h[:], False, True)
            r_t = rzp.tile([128, N], bf16, tag="rz")
            nc.scalar.activation(r_t[:], p_r[:], Act.Sigmoid)
            pr["r_t"] = r_t

        def stage_n1(pr, t):
            """hgn = W_hn h into the n-gate psum bank (reused below)."""
            h = pr["h"]
            p_n = pn.tile([128, N], f32, tag="pn")
            if h is not None:
                mm(p_n[:], bd_g[2], h[:], True, True)
            pr["p_n"] = p_n

        def stage_z(pr, t):
            """z gate in the bank stage_r freed."""
            p_z = prz.tile([128, N], f32, tag="prz")
            h = pr["h"]
            mm(p_z[:], x2_g[1], pr["xc"], True, h is None)
            if h is not None:
                mm(p_z[:], bd_g[1], h[:], False, True)
            z_t = zp.tile([128, N], bf16, tag="z")
            nc.scalar.activation(z_t[:], p_z[:], Act.Sigmoid)
            pr["z_t"] = z_t

        def stage_m(pr, t):
            """m1 = (hgn + b_hh_n)*r; x_n start=True resets the same bank;
            identity matmul accumulates m1; tanh."""
            p_n, r_t = pr["p_n"], pr["r_t"]
            m1 = m1p.tile([128, N], bf16, tag="m1")
            if pr["h"] is not None:
                nc.vector.scalar_tensor_tensor(m1[:], p_n[:], b2, r_t[:],
                                               Alu.add, Alu.mult)
            else:
                nc.vector.tensor_scalar(m1[:], r_t[:], b2, None, Alu.mult)
            mm(p_n[:], x2_g[2], pr["xc"], True, False)
            mm(p_n[:], i128, m1[:], False, True)
            n_t = np_.tile([128, N], bf16, tag="nt")
            nc.scalar.activation(n_t[:], p_n[:], Act.Tanh)
            pr["n_t"] = n_t

        def stage_zoff(pr, t):
            """off-chain: w = 1-z, zh = z*h."""
            z_t = pr["z_t"]
            w = wp.tile([128, N], bf16, tag="w")
            nc.vector.tensor_scalar(w[:], z_t[:], -1.0, 1.0, Alu.mult, Alu.add)
            if pr["h"] is not None:
                zh = zhp.tile([128, N], bf16, tag="zh")
                nc.vector.tensor_tensor(zh[:], z_t[:], pr["h"][:], Alu.mult)
                pr["zh"] = zh
            else:
                pr["zh"] = None
            pr["w"] = w

        def stage_c(pr, t):
            """h' = n*(1-z) + z*h   (zh precomputed off-chain)."""
            n_t, w, zh = pr["n_t"], pr["w"], pr["zh"]
            h_new = hp.tile([128, N], bf16, tag="h")
            if zh is not None:
                nw = nwp.tile([128, N], bf16, tag="nw")
                nc.vector.tensor_tensor(nw[:], n_t[:], w[:], Alu.mult)
                nc.vector.tensor_tensor(h_new[:], nw[:], zh[:], Alu.add)
            else:
                nc.vector.tensor_tensor(h_new[:], n_t[:], w[:], Alu.mult)
            pr["h"] = h_new

        def fc_out(pr, st, j):
            h = pr["h"]
            for g in range(2):
                p_l = plog.tile([2, N], f32, tag="pn")
                mm(p_l[:], wfc[64 * g:64 * (g + 1), :], h[64 * g:64 * (g + 1), :],
                   True, True)
                stg = st[0:2, j * PAIR + g * N:j * PAIR + (g + 1) * N]
                nc.vector.tensor_scalar(stg, p_l[:], bfc, None, Alu.add)

        for blk in range(NPAIR // IL):
            sbbase = blk * SB
            pairs = []
            st = stp.tile([2, SB], bf16, tag="st")
            for j in range(IL):
                pidx = blk * IL + j
                base = sbbase + j * PAIR
                xt = xt_pool.tile([128, XW], bf16, tag="xt")
                # flat contiguous DMA: 2 descriptors of 15KB
                nc.sync.dma_start(xt[0:2, :], xs_d[2 * pidx:2 * pidx + 2, :])
                pairs.append({"xtv": xt[:].rearrange("g (t n) -> g t n", n=N),
                              "base": base, "h": None})
            for t in range(T):
                for pr in pairs:
                    stage_r(pr, t)
                for pr in pairs:
                    stage_n1(pr, t)
                for pr in pairs:
                    stage_z(pr, t)
                for pr in pairs:
                    stage_m(pr, t)
                for pr in pairs:
                    stage_zoff(pr, t)
                for pr in pairs:
                    stage_c(pr, t)
            for j, pr in enumerate(pairs):
                fc_out(pr, st, j)
            nc.sync.dma_start(out_d[0:2, sbbase:sbbase + SB], st[0:2, :])

    nc.compile()
    return nc


def _host_constants(W_ih, W_hh, b_ih, b_hh, W_fc, b_fc):
    import ml_dtypes

    f32 = np.float32
    cbf = np.zeros((128, 1024), f32)
    cf = np.zeros((128, 8), f32)
    w_in = W_ih[:, 0].astype(f32)
    bias_g = [
        b_ih[0:64] + b_hh[0:64],          # r
        b_ih[64:128] + b_hh[64:128],      # z
        b_ih[128:192],                    # n (b_hh_n applied inside r* via b2)
    ]
    for g in range(3):
        W = W_hh[64 * g:64 * (g + 1)].astype(f32)          # [64, 64]
        cbf[0:64, 128 * g:128 * g + 64] = W.T
        cbf[64:128, 128 * g + 64:128 * g + 128] = W.T
        wg = w_in[64 * g:64 * (g + 1)]
        cbf[0, 514 + 128 * g:514 + 128 * g + 64] = wg
        cbf[1, 514 + 128 * g + 64:514 + 128 * g + 128] = wg
        cbf[2, 514 + 128 * g:514 + 128 * (g + 1)] = np.concatenate([bias_g[g]] * 2)
    cbf[:, 384:512] = np.eye(128, dtype=f32)
    cbf[0:64, 512:514] = W_fc.T
    cbf[64:128, 512:514] = W_fc.T
    cf[:, 3] = np.concatenate([b_hh[128:192]] * 2)
    cf[0:2, 4] = b_fc
    return {"cbf": cbf.astype(ml_dtypes.bfloat16), "cf": cf}


def kernel(x, W_ih, W_hh, b_ih, b_hh, W_fc, b_fc, _trace=False, _trace_kwargs=None):
    from concourse.bass_utils import run_bass_kernel_spmd

    if "nc" not in _cache:
        _cache["nc"] = _build()
    nc = _cache["nc"]

    consts = _host_constants(W_ih, W_hh, b_ih, b_hh, W_fc, b_fc)
    import ml_dtypes
    consts["ones"] = np.ones((1, XW), dtype=ml_dtypes.bfloat16)
    x = np.ascontiguousarray(np.asarray(x, np.float32))
    in_maps = []
    for c in range(NCORES):
        m = {"x": x[c * BC:(c + 1) * BC]}
        m.update(consts)
        in_maps.append(m)
    kw = {}
    if _trace:
        kw["trace"] = True
        if _trace_kwargs:
            kw.update(_trace_kwargs)
    res = run_bass_kernel_spmd(nc, in_maps, list(range(NCORES)), **kw)
    out = np.concatenate(
        [np.asarray(res.results[c]["out"]).astype(np.float32).T
         for c in range(NCORES)], axis=0)
    if _trace:
        return out, res
    return out


if __name__ == "__main__":
    rng = np.random.default_rng(0)
    s = 1.0 / np.sqrt(H)
    inputs = {
        "x": rng.standard_normal((B, T), dtype=np.float32),
        "W_ih": rng.uniform(-s, s, (3 * H, 1)).astype(np.float32),
        "W_hh": rng.uniform(-s, s, (3 * H, H)).astype(np.float32),
        "b_ih": rng.uniform(-s, s, (3 * H,)).astype(np.float32),
        "b_hh": rng.uniform(-s, s, (3 * H,)).astype(np.float32),
        "W_fc": rng.uniform(-s, s, (2, H)).astype(np.float32),
        "b_fc": rng.uniform(-s, s, (2,)).astype(np.float32),
    }
    out = kernel(**inputs)
    print(out.shape, out.dtype, out[:4])


# revision 48
# speedup vs baseline: 1.0588x; 1.0064x over previous
"""Trainium2 Bass kernel for nn_LungCancerGRU (GRU H=64, T=15, B=262144 -> logits [B,2]).

Data parallel over 8 NeuronCores (batch sharded, 32768 rows/core).

Per-core layout: gate units on SBUF partitions, batch on the free dimension.
Batch runs in pair-tiles of 1024 rows = two groups (A, B) of N=512; group A
occupies partitions 0..63, group B 64..127 of every [128, 512] tile.  Two
pair-tiles (IL=2) run in lockstep to hide the recurrence critical path.

All matmuls are bf16 (moving operand dtype determines PE rate; fp32 moving
data costs 4 cycles/column).  x is cast to bf16 once in a wide 128-partition
layout and round-tripped through scratch DRAM so the per-pair transposed
loads are 2 contiguous descriptors instead of 1024 60-byte ones.

Per timestep t (per pair-tile):
  p_rz[:, :512] = BD(W_hr^T) @ h + x2_r @ x_t       (r preact, K=128 + K=2)
  p_rz[:, 512:] = BD(W_hz^T) @ h + x2_z @ x_t       (z preact)
  p_hgn         = BD(W_hn^T) @ h                    (h-part of n gate)
  p_n           = x2_n @ x_t                        (x-part of n gate)
  r   = sigmoid(p_rz[:, :512] + bias_r)             ACT, per-partition bias
  z   = sigmoid(p_rz[:, 512:] + bias_z)
  m1  = (p_hgn + b_hh_n) * r                        DVE scalar_tensor_tensor
  p_n += I128 @ m1                                  identity-matmul accumulate
  n   = tanh(p_n + b_ih_n)                          ACT
  w = 1-z; zh = z*h; nw = n*w; h' = nw + zh         DVE bf16

FC head: logitsT [2, 512] per group via PE (stationary W_fc^T slice), bias
added in the PSUM->SBUF tensor_scalar copy, staged per-superblock and DMA'd
to a transposed [2, BC] bf16 DRAM output; the host transposes back.
"""

import sys

import numpy as np

sys.path.insert(0, "/opt/trn_rl_repo")

B, T, H = 262144, 15, 64
NCORES = 8
BC = B // NCORES          # 32768 rows per core
N = 512                   # batch columns per group
PAIR = 2 * N              # 1024 rows per pair-tile
NPAIR = BC // PAIR        # 32 pair-tiles per core
IL = 4                    # pair-tiles processed in lockstep
XW = T * N                # xt tile free width (7680)
SB = IL * PAIR            # rows per superblock (2048)

_cache = {}


def _build():
    from contextlib import ExitStack

    import concourse.bacc as bacc
    import concourse.mybir as mybir
    from concourse.tile import TileContext

    f32 = mybir.dt.float32
    bf16 = mybir.dt.bfloat16
    Act = mybir.ActivationFunctionType
    Alu = mybir.AluOpType

    nc = bacc.Bacc(None)

    x_in = nc.dram_tensor("x", [BC, T], f32, kind="ExternalInput")
    out_d = nc.dram_tensor("out", [2, BC], bf16, kind="ExternalOutput")
    cbf_in = nc.dram_tensor("cbf", [128, 1024], bf16, kind="ExternalInput")
    ones_in = nc.dram_tensor("ones", [1, XW], bf16, kind="ExternalInput")
    cf_in = nc.dram_tensor("cf", [128, 8], f32, kind="ExternalInput")
    # scratch x, bf16, t-major per 512-row group-block: xs[r, t*512+n] = x[512r+n, t]
    xs_d = nc.dram_tensor("xs", [BC // N, XW], bf16, kind="Internal")

    with TileContext(nc) as tc, ExitStack() as es:
        # ---- constants ----
        cpool = es.enter_context(tc.tile_pool(name="const", bufs=1))
        cbf = cpool.tile([128, 1024], bf16)
        nc.sync.dma_start(cbf[:], cbf_in[:])
        cf = cpool.tile([128, 8], f32)
        nc.sync.dma_start(cf[:], cf_in[:])

        bd_g = [cbf[:, 128 * g:128 * (g + 1)] for g in range(3)]
        i128 = cbf[:, 384:512]
        wfc = cbf[:, 512:514]
        # full-height x lhsT: row0/1 = per-group w_in masks, row2 = bias,
        # rows 3..127 zero.  K=128 keeps the PE array duty high (HAM stays
        # at full clock; low-K matmuls make it throttle).
        x2_g = [cbf[:, 514 + 128 * g:514 + 128 * (g + 1)] for g in range(3)]
        b2 = cf[:, 3:4]
        bfc = cf[0:2, 4:5]

        # ---- x pre-pass: f32 wide load -> transposing bf16 cast -> scratch ----
        xpre = es.enter_context(tc.tile_pool(name="xpre", bufs=1))
        xw = xpre.tile([64, XW], f32)
        nc.sync.dma_start(xw[:], x_in[:].rearrange("(p n) t -> p (n t)", p=64))
        xbw = xpre.tile([64, XW], bf16)
        nc.vector.tensor_copy(xbw[:].rearrange("p (t n) -> p t n", n=N),
                              xw[:].rearrange("p (n t) -> p t n", t=T))
        nc.sync.dma_start(xs_d[:], xbw[:])

        # ---- pools ----
        xt_pool = es.enter_context(tc.tile_pool(name="xt", bufs=5))
        # prime the xt buffers once: rows 2..127 never rewritten in-loop
        # (DMA fills rows 0..1 only); row 2 = ones carries the bias rows.
        for i in range(5):
            xtp = xt_pool.tile([128, XW], bf16, tag="xt", name=f"xtprime{i}")
            nc.gpsimd.memset(xtp[:], 0.0)
            nc.sync.dma_start(xtp[2:3, :], ones_in[:])
        hp = es.enter_context(tc.tile_pool(name="h", bufs=IL + 2))
        rzp = es.enter_context(tc.tile_pool(name="rz", bufs=IL + 2))
        zp = es.enter_context(tc.tile_pool(name="z", bufs=IL + 2))
        m1p = es.enter_context(tc.tile_pool(name="m1", bufs=IL + 2))
        np_ = es.enter_context(tc.tile_pool(name="nt", bufs=IL + 2))
        wp = es.enter_context(tc.tile_pool(name="w", bufs=IL + 2))
        zhp = es.enter_context(tc.tile_pool(name="zh", bufs=IL + 2))
        nwp = es.enter_context(tc.tile_pool(name="nw", bufs=IL + 2))
        stp = es.enter_context(tc.tile_pool(name="stage", bufs=2))
        prz = es.enter_context(tc.tile_pool(name="prz", bufs=4, space="PSUM"))
        pn = es.enter_context(tc.tile_pool(name="pn", bufs=4, space="PSUM"))
        plog = pn  # FC logits rotate through the pn slots (shared tag)

        def mm(out, lhsT, rhs, start, stop):
            nc.tensor.matmul(out, lhsT, rhs, start=start, stop=stop,
                             skip_group_check=True)

        # ---- engine warm-ups: fold const-DMA sems into each engine's clock
        pwarm = plog.tile([2, 2], f32, tag="pn")
        mm(pwarm[:], cbf[0:2, 0:2], cbf[0:2, 0:2], True, True)
        wt = cpool.tile([2, 8], f32)
        nc.vector.tensor_copy(wt[0:1, 0:1], cf[0:1, 0:1])
        nc.vector.tensor_copy(wt[0:1, 1:2], cbf[0:1, 0:1])
        nc.scalar.copy(wt[0:1, 2:3], cf[0:1, 0:1])
        nc.scalar.copy(wt[0:1, 3:4], cbf[0:1, 0:1])

        def stage_r(pr, t):
            """r gate: x matmul (start) + recurrent matmul (stop), sigmoid.
            One 1-bank psum tile, freed at the sigmoid -> z reuses it."""
            xcols = pr["xtv"][:, t, :]
            pr["xc"] = xcols
            p_r = prz.tile([128, N], f32, tag="prz")
            h = pr["h"]
            mm(p_r[:], x2_g[0], xcols, True, h is None)
            if h is not None:
                mm(p_r[:], bd_g[0], h[:], False, True)
            r_t = rzp.tile([128, N], bf16, tag="rz")
            nc.scalar.activation(r_t[:], p_r[:], Act.Sigmoid)
            pr["r_t"] = r_t

        def stage_n1(pr, t):
            """hgn = W_hn h into the n-gate psum bank (reused below)."""
            h = pr["h"]
            p_n = pn.tile([128, N], f32, tag="pn")
            if h is not None:
                mm(p_n[:], bd_g[2], h[:], True, True)
            pr["p_n"] = p_n

        def stage_z(pr, t):
            """z gate in the bank stage_r freed."""
            p_z = prz.tile([128, N], f32, tag="prz")
            h = pr["h"]
            mm(p_z[:], x2_g[1], pr["xc"], True, h is None)
            if h is not None:
                mm(p_z[:], bd_g[1], h[:], False, True)
            z_t = zp.tile([128, N], bf16, tag="z")
            nc.scalar.activation(z_t[:], p_z[:], Act.Sigmoid)
            pr["z_t"] = z_t

        def stage_m(pr, t):
            """m1 = (hgn + b_hh_n)*r; x_n start=True resets the same bank;
            identity matmul accumulates m1; tanh."""
            p_n, r_t = pr["p_n"], pr["r_t"]
            m1 = m1p.tile([128, N], bf16, tag="m1")
            if pr["h"] is not None:
                nc.vector.scalar_tensor_tensor(m1[:], p_n[:], b2, r_t[:],
                                               Alu.add, Alu.mult)
            else:
                nc.vector.tensor_scalar(m1[:], r_t[:], b2, None, Alu.mult)
            mm(p_n[:], x2_g[2], pr["xc"], True, False)
            mm(p_n[:], i128, m1[:], False, True)
            n_t = np_.tile([128, N], bf16, tag="nt")
            nc.scalar.activation(n_t[:], p_n[:], Act.Tanh)
            pr["n_t"] = n_t

        def stage_zoff(pr, t):
            """off-chain: w = 1-z, zh = z*h."""
            z_t = pr["z_t"]
            w = wp.tile([128, N], bf16, tag="w")
            nc.vector.tensor_scalar(w[:], z_t[:], -1.0, 1.0, Alu.mult, Alu.add)
            if pr["h"] is not None:
                zh = zhp.tile([128, N], bf16, tag="zh")
                nc.vector.tensor_tensor(zh[:], z_t[:], pr["h"][:], Alu.mult)
                pr["zh"] = zh
            else:
                pr["zh"] = None
            pr["w"] = w

        def stage_c(pr, t):
            """h' = n*(1-z) + z*h   (zh precomputed off-chain)."""
            n_t, w, zh = pr["n_t"], pr["w"], pr["zh"]
            h_new = hp.tile([128, N], bf16, tag="h")
            if zh is not None:
                nw = nwp.tile([128, N], bf16, tag="nw")
                nc.vector.tensor_tensor(nw[:], n_t[:], w[:], Alu.mult)
                nc.vector.tensor_tensor(h_new[:], nw[:], zh[:], Alu.add)
            else:
                nc.vector.tensor_tensor(h_new[:], n_t[:], w[:], Alu.mult)
            pr["h"] = h_new

        def fc_out(pr, st, j):
            h = pr["h"]
            for g in range(2):
                p_l = plog.tile([2, N], f32, tag="pn")
                mm(p_l[:], wfc[64 * g:64 * (g + 1), :], h[64 * g:64 * (g + 1), :],
                   True, True)
                stg = st[0:2, j * PAIR + g * N:j * PAIR + (g + 1) * N]
                nc.vector.tensor_scalar(stg, p_l[:], bfc, None, Alu.add)

        for blk in range(NPAIR // IL):
            sbbase = blk * SB
            pairs = []
            st = stp.tile([2, SB], bf16, tag="st")
            for j in range(IL):
                pidx = blk * IL + j
                base = sbbase + j * PAIR
                xt = xt_pool.tile([128, XW], bf16, tag="xt")
                # flat contiguous DMA: 2 descriptors of 15KB
                nc.sync.dma_start(xt[0:2, :], xs_d[2 * pidx:2 * pidx + 2, :])
                pairs.append({"xtv": xt[:].rearrange("g (t n) -> g t n", n=N),
                              "base": base, "h": None})
            for t in range(T):
                for pr in pairs:
                    stage_r(pr, t)
                for pr in pairs:
                    stage_n1(pr, t)
                for pr in pairs:
                    stage_z(pr, t)
                for pr in pairs:
                    stage_m(pr, t)
                for pr in pairs:
                    stage_zoff(pr, t)
                for pr in pairs:
                    stage_c(pr, t)
            for j, pr in enumerate(pairs):
                fc_out(pr, st, j)
            nc.sync.dma_start(out_d[0:2, sbbase:sbbase + SB], st[0:2, :])

    nc.compile()
    return nc


def _host_constants(W_ih, W_hh, b_ih, b_hh, W_fc, b_fc):
    import ml_dtypes

    f32 = np.float32
    cbf = np.zeros((128, 1024), f32)
    cf = np.zeros((128, 8), f32)
    w_in = W_ih[:, 0].astype(f32)
    bias_g = [
        b_ih[0:64] + b_hh[0:64],          # r
        b_ih[64:128] + b_hh[64:128],      # z
        b_ih[128:192],                    # n (b_hh_n applied inside r* via b2)
    ]
    for g in range(3):
        W = W_hh[64 * g:64 * (g + 1)].astype(f32)          # [64, 64]
        cbf[0:64, 128 * g:128 * g + 64] = W.T
        cbf[64:128, 128 * g + 64:128 * g + 128] = W.T
        wg = w_in[64 * g:64 * (g + 1)]
        cbf[0, 514 + 128 * g:514 + 128 * g + 64] = wg
        cbf[1, 514 + 128 * g + 64:514 + 128 * g + 128] = wg
        cbf[2, 514 + 128 * g:514 + 128 * (g + 1)] = np.concatenate([bias_g[g]] * 2)
    cbf[:, 384:512] = np.eye(128, dtype=f32)
    cbf[0:64, 512:514] = W_fc.T
    cbf[64:128, 512:514] = W_fc.T
    cf[:, 3] = np.concatenate([b_hh[128:192]] * 2)
    cf[0:2, 4] = b_fc
    return {"cbf": cbf.astype(ml_dtypes.bfloat16), "cf": cf}


def kernel(x, W_ih, W_hh, b_ih, b_hh, W_fc, b_fc, _trace=False, _trace_kwargs=None):
    from concourse.bass_utils import run_bass_kernel_spmd

    if "nc" not in _cache:
        _cache["nc"] = _build()
    nc = _cache["nc"]

    consts = _host_constants(W_ih, W_hh, b_ih, b_hh, W_fc, b_fc)
    import ml_dtypes
    consts["ones"] = np.ones((1, XW), dtype=ml_dtypes.bfloat16)
    x = np.ascontiguousarray(np.asarray(x, np.float32))
    in_maps = []
    for c in range(NCORES):
        m = {"x": x[c * BC:(c + 1) * BC]}
        m.update(consts)
        in_maps.append(m)
    kw = {}
    if _trace:
        kw["trace"] = True
        if _trace_kwargs:
            kw.update(_trace_kwargs)
    res = run_bass_kernel_spmd(nc, in_maps, list(range(NCORES)), **kw)
    out = np.concatenate(
        [np.asarray(res.results[c]["out"]).astype(np.float32).T
         for c in range(NCORES)], axis=0)
    if _trace:
        return out, res
    return out


if __name__ == "__main__":
    rng = np.random.default_rng(0)
    s = 1.0 / np.sqrt(H)
    inputs = {
        "x": rng.standard_normal((B, T), dtype=np.float32),
        "W_ih": rng.uniform(-s, s, (3 * H, 1)).astype(np.float32),
        "W_hh": rng.uniform(-s, s, (3 * H, H)).astype(np.float32),
        "b_ih": rng.uniform(-s, s, (3 * H,)).astype(np.float32),
        "b_hh": rng.uniform(-s, s, (3 * H,)).astype(np.float32),
        "W_fc": rng.uniform(-s, s, (2, H)).astype(np.float32),
        "b_fc": rng.uniform(-s, s, (2,)).astype(np.float32),
    }
    out = kernel(**inputs)
    print(out.shape, out.dtype, out[:4])
